# revision 4
# baseline (speedup 1.0000x reference)
# Bahdanau-attention kernel for TRN2, data-parallel over batch across 8 NeuronCores.
#
# reference math (B=16, S=2048, H=1024):
#   h_proj = hidden @ W[:, :H].T                      [B, H]
#   e_proj = einsum('bsh,gh->bsg', enc, W[:, H:])     [B, S, H]
#   scores = tanh(h_proj[:,None,:] + e_proj + b)      [B, S, H]
#   logits = scores @ v                               [B, S]
#   out    = softmax(logits, -1)[:, None, :]          [B, 1, S]
#
# Per-core layout (2 batches/core). The e_proj GEMM is the hard floor:
# 256 DoubleRow fp8 MMs x 512 cols = 55.3us at the 157 TF/s fp8 peak.
# Everything else hides under it:
#   - h_proj + b precomputed on host (hb), uploaded as a [128, GT, BPC] bias
#   - tanh on ScalarE per (chunk, j) with per-partition bias, fp8 scores out
#   - v-dot deferred per batch: 16 DoubleRow MMs, chunks col-grouped to
#     partition strips 32c of ONE psum bank (concurrent col-group execution);
#     v is padded to 32 cols (col 0 = 16*v, rest 0) so the whole bank is
#     written and exp() sees no stale garbage
#   - softmax: one Exp over [128,512] with accum_out, mask-matmul broadcasts
#     the partition-sum to all lanes, DVE reciprocal+scale on 128 lanes
#   - HAM warm-up: dummy MMs on memset junk while the first DMAs land, and a
#     tiny tanh to preload the ACT table set before the first real tanh
#   - DMA kicks split across Sync/GpSimd queues so weight/enc transfers start
#     as soon as the NEFF preamble ends
# Softmax skips max-subtraction: |logits| <= ||v||_1 * max|tanh| ~ 16, so
# exp() cannot overflow in f32 and the softmax ratio is unchanged.

import numpy as np
import ml_dtypes

import concourse.bass as bass
import concourse.mybir as mybir
import concourse.tile as tile
from concourse import bacc
from concourse.bass_utils import run_bass_kernel_spmd

B, S, H = 16, 2048, 1024
NCORES = 8
BPC = B // NCORES          # batches per core
KT = H // 128              # contraction tiles
GT = H // 128              # output (g) tiles
SBLK = 512                 # s-chunk (one PSUM bank of f32)
NSB = S // SBLK

BF16 = mybir.dt.bfloat16
F32 = mybir.dt.float32
FP8 = mybir.dt.float8e4
WSCALE = 32.0              # W_e pre-scaled into fp8's sweet range; undone in tanh's scale
VSCALE = 16.0              # v pre-scaled; undone in exp's scale
DR = mybir.MatmulPerfMode.DoubleRow

_CACHE = {}


def _build():
    nc = bacc.Bacc("TRN2", target_bir_lowering=False, debug=False, num_devices=NCORES)

    encT_d = nc.dram_tensor("encT", [BPC, 128, KT, S], FP8, kind="ExternalInput")
    we_d = nc.dram_tensor("we", [128, GT, KT, 128], FP8, kind="ExternalInput")
    hb_d = nc.dram_tensor("hb", [128, GT, BPC], F32, kind="ExternalInput")
    vvec_d = nc.dram_tensor("vvec", [128, GT, 32], FP8, kind="ExternalInput")
    mask_d = nc.dram_tensor("mask4", [128, 128], F32, kind="ExternalInput")
    out_d = nc.dram_tensor("out", [BPC, NSB, SBLK], F32, kind="ExternalOutput")

    ACT = mybir.ActivationFunctionType

    with tile.TileContext(nc) as tc:
        with (
            tc.tile_pool(name="const", bufs=1) as constp,
            tc.tile_pool(name="wp", bufs=1) as wp,
            tc.tile_pool(name="encp", bufs=1) as encp,
            tc.tile_pool(name="scp", bufs=2) as scp,
            tc.tile_pool(name="smallp", bufs=2) as smallp,
            tc.tile_pool(name="mps", bufs=4, space="PSUM") as mps,
            tc.tile_pool(name="lgp", bufs=2, space="PSUM") as lgp,
        ):
            # --- warm-up: junk memset feeds (a) an ACT-table-preloading tanh
            # and (b) dummy MMs that lift HAM to K=8/8 before real data lands.
            junk = constp.tile([128, 2, 128], FP8, tag="junk")
            nc.gpsimd.memset(junk[:], 0)
            jact = constp.tile([1, 1], F32, tag="jact")
            nc.scalar.activation(jact[:], junk[0:1, 0, 0:1], ACT.Tanh)
            dummy = lgp.tile([128, SBLK], F32, tag="lg", name="dummy")
            for _ in range(12):
                nc.tensor.matmul(
                    dummy[:, 0:128], junk[:], junk[:],
                    start=True, stop=True, perf_mode=DR,
                )

            # --- DMA kicks. Two queues issue in parallel; within each queue
            # the order is what the compute loop consumes first.
            we_sb = [None] * GT

            def load_we(j, eng):
                t = wp.tile([128, KT, 128], FP8, name=f"we{j}", tag=f"we{j}")
                eng.dma_start(out=t[:], in_=we_d[:, j])
                we_sb[j] = t

            enc_sb = [
                encp.tile([128, KT, S], FP8, name=f"enc{bb}", tag=f"enc{bb}")
                for bb in range(BPC)
            ]

            def load_enc(bb, lo, size, eng):
                sl = slice(lo, lo + size)
                eng.dma_start(out=enc_sb[bb][:, :, sl], in_=encT_d[bb][:, :, sl])

            hb_sb = constp.tile([128, GT, BPC], F32, tag="hb")
            v_sb = constp.tile([128, GT, 32], FP8, tag="vvec")
            mask_sb = constp.tile([128, 128], F32, tag="mask4")

            # Sync queue: first enc chunk, even-j weights, hb, remaining enc.
            load_enc(0, 0, SBLK, nc.sync)
            load_we(0, nc.sync)
            load_we(2, nc.sync)
            load_we(4, nc.sync)
            nc.sync.dma_start(out=hb_sb[:], in_=hb_d[:])
            load_enc(0, SBLK, SBLK, nc.sync)
            load_we(6, nc.sync)
            load_enc(0, 2 * SBLK, SBLK, nc.sync)
            load_enc(0, 3 * SBLK, SBLK, nc.sync)
            load_enc(1, 0, 2 * SBLK, nc.sync)
            load_enc(1, 2 * SBLK, 2 * SBLK, nc.sync)
            # GpSimd queue: odd-j weights + small constants.
            load_we(1, nc.gpsimd)
            load_we(3, nc.gpsimd)
            load_we(5, nc.gpsimd)
            load_we(7, nc.gpsimd)
            nc.gpsimd.dma_start(out=v_sb[:], in_=vvec_d[:])
            nc.gpsimd.dma_start(out=mask_sb[:], in_=mask_d[:])

            # --- main loop ---
            sc_tiles = {}

            def emit_tail(bb):
                # v-dot: jp-outer / chunk-inner so consecutive MMs hit the 4
                # distinct col-groups (partition strips 32c of one bank) and
                # overlap on the PE array. Only the very first MM clears the
                # bank (start=True); each strip's first write then lands on
                # cleared has_written bits and overwrites correctly.
                # DoubleRow rejects dst partitions != 0 (s3d3 ISA check), so
                # these run as plain fp8 MMs (same 1 col/cycle streaming rate)
                # — and the col-group overlap more than pays for the extra
                # contraction passes.
                lg = lgp.tile([128, SBLK], F32, tag="lg", name=f"lg{bb}")
                first = True
                for j in range(GT):
                    for c in range(NSB):
                        nc.tensor.matmul(
                            lg[32 * c : 32 * c + 32, :],
                            v_sb[:, j, :],
                            sc_tiles[(bb, c, j // 2)][:, j % 2, :],
                            start=first,
                            stop=(j == GT - 1 and c == NSB - 1),
                            skip_group_check=True,
                            tile_position=(0, 32 * c),
                        )
                        first = False
                exps = smallp.tile([128, SBLK], F32, tag="exps", name=f"exps{bb}")
                acc = smallp.tile([128, 1], F32, tag="acc", name=f"acc{bb}")
                nc.scalar.activation(
                    exps[:], lg[:], ACT.Exp, accum_out=acc[:], scale=1.0 / VSCALE
                )
                # Broadcast the 4 strip-sums to every partition: ones at rows
                # {0,32,64,96} of the stationary pick out the real sums.
                nc.tensor.matmul(
                    lg[:, 0:1], mask_sb[:], acc[:],
                    start=True, stop=True, skip_group_check=True,
                )
                rsum = smallp.tile([128, 1], F32, tag="rsum", name=f"rsum{bb}")
                nc.vector.reciprocal(rsum[:], lg[:, 0:1])
                outsb = smallp.tile([128, SBLK], F32, tag="outsb", name=f"outsb{bb}")
                nc.vector.tensor_scalar_mul(outsb[:], exps[:], rsum[:])
                nc.sync.dma_start(out=out_d[bb], in_=outsb[0:128:32, :])

            for bb in range(BPC):
                for c in range(NSB):
                    # previous batch's tail goes here, one chunk into this
                    # batch, so its v-dot never head-blocks the PE queue.
                    if bb == 1 and c == 1:
                        emit_tail(0)
                    sl = slice(c * SBLK, (c + 1) * SBLK)
                    for jp in range(GT // 2):
                        sc2 = scp.tile(
                            [128, 2, SBLK], FP8,
                            name=f"sc{bb}_{c}_{jp}", tag=f"sc_{c}_{jp}",
                        )
                        sc_tiles[(bb, c, jp)] = sc2
                        for half in range(2):
                            j = 2 * jp + half
                            mp = mps.tile([128, SBLK], F32, tag="mp", name=f"mp{j}")
                            for kp in range(KT // 2):
                                nc.tensor.matmul(
                                    mp[:],
                                    we_sb[j][:, 2 * kp : 2 * kp + 2, :],
                                    enc_sb[bb][:, 2 * kp : 2 * kp + 2, sl],
                                    start=(kp == 0),
                                    stop=(kp == KT // 2 - 1),
                                    perf_mode=DR,
                                )
                            nc.scalar.activation(
                                sc2[:, half, :], mp[:], ACT.Tanh,
                                bias=hb_sb[:, j, bb : bb + 1],
                                scale=1.0 / WSCALE,
                            )
            emit_tail(1)

    nc.compile()
    return nc


def _get_nc():
    if "nc" not in _CACHE:
        _CACHE["nc"] = _build()
    return _CACHE["nc"]


def _make_in_maps(hidden, encoder_outputs, W, b, v):
    bf = ml_dtypes.bfloat16
    fp8 = ml_dtypes.float8_e4m3
    WT = np.ascontiguousarray(W.T)  # [2H, H]; WT[hin, gout]
    w_tiles = WT.reshape(2, KT, 128, GT, 128).transpose(0, 2, 3, 1, 4)  # [half, p, j, k, m]
    we_host = np.ascontiguousarray(w_tiles[1] * WSCALE).astype(fp8)

    # h_proj + b on host in f64: a [B, H] bias, 0.03% of the FLOPs.
    hb_full = (
        hidden.astype(np.float64) @ W[:, :H].astype(np.float64).T
        + b.astype(np.float64)
    ).astype(np.float32)  # [B, H]

    v_host = np.zeros((128, GT, 32), dtype=fp8)
    v_host[:, :, 0] = (v.reshape(GT, 128).T * VSCALE).astype(fp8)

    mask_host = np.zeros((128, 128), dtype=np.float32)
    mask_host[0::32, :] = 1.0

    in_maps = []
    for i in range(NCORES):
        hs = hb_full[BPC * i : BPC * (i + 1)]  # [BPC, H]
        hb_host = np.ascontiguousarray(
            hs.reshape(BPC, GT, 128).transpose(2, 1, 0)
        ).astype(np.float32)  # [128, GT, BPC]
        es = encoder_outputs[BPC * i : BPC * (i + 1)]  # [BPC, S, H]
        # [bb, p, k, s]: partition dim outermost so one DMA fills all k-tiles
        # of a column range with matching AP iteration order
        eT = np.ascontiguousarray(
            es.transpose(0, 2, 1).reshape(BPC, KT, 128, S).transpose(0, 2, 1, 3)
        ).astype(fp8)
        in_maps.append(
            {
                "encT": eT,
                "we": we_host,
                "hb": hb_host,
                "vvec": v_host,
                "mask4": mask_host,
            }
        )
    return in_maps


def _run(in_maps, **kwargs):
    nc = _get_nc()
    try:
        return run_bass_kernel_spmd(
            nc, in_maps, core_ids=list(range(NCORES)), **kwargs
        )
    except Exception:
        # A first execution right after NEFF load has been seen to wedge the
        # device once; it recovers after a short pause. Retry once.
        import time as _time

        _time.sleep(20)
        return run_bass_kernel_spmd(
            nc, in_maps, core_ids=list(range(NCORES)), **kwargs
        )


def kernel(hidden, encoder_outputs, W, b, v):
    hidden = np.asarray(hidden, dtype=np.float32)
    encoder_outputs = np.asarray(encoder_outputs, dtype=np.float32)
    W = np.asarray(W, dtype=np.float32)
    b = np.asarray(b, dtype=np.float32)
    v = np.asarray(v, dtype=np.float32)

    in_maps = _make_in_maps(hidden, encoder_outputs, W, b, v)
    res = _run(in_maps)
    outs = [
        np.asarray(res.results[i]["out"], dtype=np.float32).reshape(BPC, S)
        for i in range(NCORES)
    ]
    return np.concatenate(outs, axis=0).reshape(B, 1, S)
